# revision 30
# baseline (speedup 1.0000x reference)
"""Deformable transformer encoder layer on 8 Trainium2 NeuronCores.

Sharding: core c handles batch c//4, query-quarter c%4 (3840 queries each).

v3 software-pipelined design:
  - Host-permuted layouts put h innermost (stride 1) everywhere so DVE
    broadcast APs hit the 2x_1p packed mode on the big sampling multiplies;
    hat-weight chain in fp16; value table columns (d,h).
  - Phase A (value projection) uses a host-transposed bf16 srcT (pure
    matmuls, no device transposes), quad 512-row iterations, one load +
    one store DMA each.
  - All 30 query tiles' pre-gather stage (S0: loads, q projection, softmax,
    positions, hat weights) is emitted interleaved with phase A, filling
    the otherwise idle vector/scalar engines during the table build.
  - Main loop is software-pipelined per tile: gathers issued one tile
    ahead; LN2+store of tile i-1 emitted between the sampling stage and
    LN1/FFN of tile i, so the vector engine never waits on the FFN chain.
"""
import os
import sys

sys.path.insert(0, '/opt/trn_rl_repo')

import dataclasses
import numpy as np
import ml_dtypes

import concourse.bass as bass
import concourse.mybir as mybir
from concourse.tile import TileContext

# ---- tile drain workaround (this walrus rejects multi-wait drains) ----
import concourse.tile as _tile_mod
from concourse.tile_sem_assignment import tick_to_sem as _tick_to_sem


def _split_drain_and_barrier(self, tick_clock, wait_clock):
    gc = tick_clock.global_clock
    allocated = self.sems.allocated() if self.sems is not None else {}
    for proc, sem in sorted(allocated.items()):
        t = gc[proc]
        if t > 0:
            self.nc.sync.wait_ge(sem, _tick_to_sem(t, proc))
    self.nc.sync.drain()
    self.nc.all_engine_barrier()
    assert self.sems is not None
    popped = self.nc._tile_sem_poison_stack.pop()
    assert popped is self._sem_poison
    self.nc.clear_and_free_semaphores(list(self.sems.allocated().values()))
    self.nc.all_engine_barrier()


_tile_mod.TileContext._drain_and_barrier = _split_drain_and_barrier

_MAX_WAITS = 1
_wsplit_n = [0]


def _split_excess_waits(nc):
    """Walrus rejects instructions with >2 sem waits; move extras to nops."""
    for f in nc.m.functions:
        for bb in f.blocks:
            out = []
            for inst in list(bb.instructions):
                si = inst.sync_info
                waits = list(si.on_wait) if (si and si.on_wait) else []
                if len(waits) > _MAX_WAITS:
                    extra = waits[:-_MAX_WAITS]
                    keep = waits[-_MAX_WAITS:]
                    for j in range(0, len(extra), _MAX_WAITS):
                        _wsplit_n[0] += 1
                        nop = mybir.InstNoOp(name=f'wsplit-{_wsplit_n[0]}',
                                             ins=[], outs=[])
                        nop.engine = inst.engine
                        nop.sync_info = mybir.SyncInfo(
                            on_wait=extra[j:j + _MAX_WAITS], on_update=[])
                        out.append(nop)
                    inst.sync_info = mybir.SyncInfo(
                        on_wait=keep, on_update=list(si.on_update or []))
                out.append(inst)
            bb.instructions = out

f32 = mybir.dt.float32
bf16 = mybir.dt.bfloat16
fp16 = mybir.dt.float16
i32 = mybir.dt.int32
AF = mybir.ActivationFunctionType
OP = mybir.AluOpType

# ---- problem constants ----
D = 256
DF = 1024
H = 8
L = 4
NP = 4
LEVEL_LENS = (8192, 4096, 2048, 1024)
LEN_IN = 15360
N_CORES = 8
EPS = 1e-5

W = 6
PAD = 8
_starts = []
_acc = PAD
for _t in LEVEL_LENS:
    _starts.append(_acc)
    _acc += _t + PAD
PSTARTS = tuple(_starts)
VROWS = _acc               # 15400
QC = LEN_IN // 4           # 3840
NT_Q = QC // 128           # 30
WIN = W * D                # 1536
HLP = H * L * NP           # 128
WH = W * H                 # 48
LRH = L * W * H            # 192


def _bc(ap, dims):
    """Replace the free dims of a 2-d AP with an explicit dim list."""
    return dataclasses.replace(ap, ap=[list(ap.ap[0])] + [list(d) for d in dims])


def _off(ap, delta):
    return dataclasses.replace(ap, offset=ap.offset + delta)


def _build_program():
    nc = bass.Bass(trn_type='TRN2')

    din = {}
    def I(name, shape, dt):
        din[name] = nc.dram_tensor(name, shape, dt, kind='ExternalInput')
        return din[name]

    srcT = I('srcT', [D, LEN_IN], bf16)      # host-transposed bf16 src
    srcq = I('srcq', [QC, D], f32)
    pos_q = I('pos_q', [QC, D], f32)
    te_q = I('te_q', [QC, D], f32)
    ref_q = I('ref_q', [QC, L], f32)
    wval = I('wval', [128, 2 * D], bf16)     # k-chunk kc at cols [kc*D,(kc+1)*D], out cols (d,h)
    bval = I('bval', [1, 2 * D], bf16)       # (d,h) order, tiled x2
    woa = I('woa', [128, 2 * D], bf16)       # off cols (lv,p,h), attn cols (lv,p,h)
    boa = I('boa', [1, D], bf16)             # concat(b_off, b_attn) permuted
    wout = I('wout', [128, 2 * D], bf16)     # rows (d,h)-permuted
    bout = I('bout', [1, D], bf16)
    w1 = I('w1', [128, 2 * DF], bf16)        # chunk kc at cols [kc*DF,(kc+1)*DF]
    b1t = I('b1t', [128, 8], f32)            # b1 transposed: [dffn%128, dffn//128]
    w2 = I('w2', [128, 8 * D], bf16)         # chunk k at cols [k*D,(k+1)*D]
    b2 = I('b2', [1, D], bf16)
    ln1g = I('ln1g', [128, D], bf16)         # replicated over partitions
    ln1b = I('ln1b', [128, D], f32)
    ln2g = I('ln2g', [128, D], bf16)
    ln2b = I('ln2b', [128, D], f32)
    ident = I('ident', [128, 128], bf16)
    ones1 = I('ones1', [1, 128], bf16)
    riota = I('riota', [128, HLP * W], fp16) # col (lv,p)*48 + r*8 + h -> r
    tsc = I('tsc', [128, L], f32)            # replicated level lens
    slc = I('slc', [128, L], i32)            # replicated PSTARTS[l] - 18

    out_q = nc.dram_tensor('out_q', [QC, D], f32, kind='ExternalOutput')

    with TileContext(nc) as tc:
        with tc.tile_pool(name='cst', bufs=1) as cst, \
             tc.tile_pool(name='io', bufs=3) as io, \
             tc.tile_pool(name='car', bufs=32) as car, \
             tc.tile_pool(name='wk', bufs=3) as wk, \
             tc.tile_pool(name='gat', bufs=3) as gat, \
             tc.tile_pool(name='psA', bufs=3, space='PSUM') as psA, \
             tc.tile_pool(name='psB', bufs=2, space='PSUM') as psB, \
             tc.tile_pool(name='ptr', bufs=3, space='PSUM') as ptr, \
             tc.tile_pool(name='dram', bufs=1, space='DRAM') as dram:

            value_dram = dram.tile([VROWS, D], bf16)

            def ctile(name, dt=bf16):
                t = cst.tile(list(din[name].shape), dt, tag=name)
                nc.sync.dma_start(t[:], din[name][:, :])
                return t

            identS = ctile('ident')
            onesS = ctile('ones1')
            riotaS = ctile('riota', fp16)
            tscS = ctile('tsc', f32)
            slcS = ctile('slc', i32)
            wvalS = ctile('wval')
            bvalS = ctile('bval')
            woaS = ctile('woa')
            boaS = ctile('boa')
            woutS = ctile('wout')
            boutS = ctile('bout')
            w1S = ctile('w1')
            b1tS = ctile('b1t', f32)
            w2S = ctile('w2')
            b2S = ctile('b2')
            ln1gS = ctile('ln1g', bf16)
            ln1bS = ctile('ln1b', f32)
            ln2gS = ctile('ln2g', bf16)
            ln2bS = ctile('ln2b', f32)

            epsS = cst.tile([128, 1], f32, tag='epsS')
            nc.vector.memset(epsS[:], EPS)
            zpad = cst.tile([PAD, D], bf16, tag='zpad')
            nc.vector.memset(zpad[:], 0.0)
            nc.sync.dma_start(value_dram[0:PAD, :], zpad[:])
            for lv in range(L):
                r0 = PSTARTS[lv] + LEVEL_LENS[lv]
                nc.sync.dma_start(value_dram[r0:r0 + PAD, :], zpad[:])

            vwin = dataclasses.replace(value_dram[:, :], ap=[[D, VROWS], [1, D]])
            st = [dict() for _ in range(NT_Q)]

            # ---- phase A quad: 512 value rows ----
            cums = [0]
            for t in LEVEL_LENS:
                cums.append(cums[-1] + t)

            def phaseA_quad(j):
                r = j * 512
                lv = next(k for k in range(L) if r < cums[k + 1])
                prow = PSTARTS[lv] + (r - cums[lv])
                sT = io.tile([128, 1024], bf16, tag='va_in')
                src_ap = dataclasses.replace(
                    srcT[:, :], offset=r,
                    ap=[[LEN_IN, 128], [LEN_IN * 128, 2], [128, 4], [1, 128]])
                nc.sync.dma_start(sT[:], src_ap)
                v_b = wk.tile([128, 1024], bf16, tag='va_out', bufs=2)
                for half in range(2):
                    vps = psB.tile([128, 512], f32, tag='p512')
                    for t4 in range(2):
                        j4 = half * 2 + t4
                        for h in range(2):
                            # start=True on the first matmul zeroes the whole
                            # 2KB zero region (both 256-col halves); everything
                            # else accumulates within the single group.
                            nc.tensor.matmul(
                                vps[:, t4 * 256:(t4 + 1) * 256],
                                sT[:, h * 512 + j4 * 128: h * 512 + (j4 + 1) * 128],
                                wvalS[:, h * D:(h + 1) * D],
                                start=(t4 == 0 and h == 0), stop=False)
                    nc.tensor.matmul(vps[:], onesS[:1, :], bvalS[:1, :],
                                     start=False, stop=True)
                    nc.scalar.activation(v_b[:, half * 512:(half + 1) * 512], vps[:], AF.Copy)
                out_ap = dataclasses.replace(
                    value_dram[:, :], offset=prow * D,
                    ap=[[D, 128], [D * 128, 4], [1, D]])
                nc.sync.dma_start(out_ap, v_b[:])

            # ---- S0: pre-gather stage for tile i ----
            def S0(i):
                rq = i * 128
                s = st[i]
                srco = io.tile([128, D], f32, tag='srco')
                nc.sync.dma_start(srco[:], srcq[rq:rq + 128, :])
                post = io.tile([128, D], f32, tag='post')
                nc.sync.dma_start(post[:], pos_q[rq:rq + 128, :])
                reft = io.tile([128, L], f32, tag='reft')
                nc.sync.dma_start(reft[:], ref_q[rq:rq + 128, :])

                qb = wk.tile([128, D], bf16, tag='qb')
                nc.gpsimd.tensor_tensor(qb[:], srco[:], post[:], OP.add)
                qT = wk.tile([128, D], bf16, tag='qT')
                for kc in range(2):
                    tp = ptr.tile([128, 128], bf16, tag='tr')
                    nc.tensor.transpose(tp[:], qb[:, kc * 128:(kc + 1) * 128], identS[:])
                    nc.scalar.activation(qT[:, kc * 128:(kc + 1) * 128], tp[:], AF.Copy)

                oaps = psA.tile([128, D], f32, tag='oaps')
                nc.tensor.matmul(oaps[:], qT[:, 0:128], woaS[:, 0:D], start=True, stop=False)
                nc.tensor.matmul(oaps[:], qT[:, 128:256], woaS[:, D:2 * D], start=False, stop=False)
                nc.tensor.matmul(oaps[:], onesS[:1, :], boaS[:1, :], start=False, stop=True)

                # softmax over (lv,p) per h; cols (lv,p,h), h innermost
                ex = wk.tile([128, 128], f32, tag='ex')
                nc.scalar.activation(ex[:], oaps[:, 128:256], AF.Exp)
                zs = wk.tile([128, 8], f32, tag='zs')
                nc.vector.tensor_reduce(zs[:], _bc(ex[:, :], [[1, H], [H, 16]]),
                                        mybir.AxisListType.X, OP.add)
                zr = wk.tile([128, 8], f32, tag='zr')
                nc.vector.reciprocal(zr[:], zs[:])
                wnh = wk.tile([128, 128], fp16, tag='wnh')
                nc.vector.tensor_tensor(
                    _bc(wnh[:, :], [[H, 16], [1, H]]),
                    _bc(ex[:, :], [[H, 16], [1, H]]),
                    _bc(zr[:, :], [[0, 16], [1, H]]), OP.mult)

                # positions
                art = wk.tile([128, L], f32, tag='art')
                nc.vector.tensor_tensor(art[:], reft[:], tscS[:, :], OP.mult)
                ar16 = wk.tile([128, L], f32, tag='ar16')
                nc.vector.tensor_scalar(ar16[:], art[:], 15.5, None, OP.add)
                fli = wk.tile([128, L], i32, tag='fli')
                nc.vector.tensor_copy(fli[:], ar16[:])
                fl16 = wk.tile([128, L], f32, tag='fl16')
                nc.vector.tensor_copy(fl16[:], fli[:])
                wri = car.tile([128, L], i32, tag='wri')
                nc.vector.tensor_tensor(wri[:], fli[:], slcS[:, :], OP.add)
                arf = wk.tile([128, L], f32, tag='arf')
                nc.vector.scalar_tensor_tensor(arf[:], ar16[:], 2.0, fl16[:], OP.add, OP.subtract)

                xq = wk.tile([128, HLP], fp16, tag='xq')
                nc.vector.tensor_tensor(
                    xq[:], oaps[:, 0:128],
                    _bc(arf[:, :], [[1, L], [0, NP], [0, H]]), OP.add)

                dd = wk.tile([128, HLP * W], fp16, tag='dd', bufs=2)
                nc.vector.tensor_tensor(
                    dd[:], riotaS[:, :],
                    _bc(xq[:, :], [[H, 16], [0, W], [1, H]]), OP.subtract)
                da = wk.tile([128, HLP * W], fp16, tag='da', bufs=2)
                nc.scalar.activation(da[:], dd[:], AF.Abs)
                h1 = wk.tile([128, HLP * W], fp16, tag='h1', bufs=2)
                nc.scalar.activation(h1[:], da[:], AF.Relu, bias=1.0, scale=-1.0)
                c3p = wk.tile([128, HLP * W], fp16, tag='c3p')
                nc.vector.tensor_tensor(
                    c3p[:], h1[:],
                    _bc(wnh[:, :], [[H, 16], [0, W], [1, H]]), OP.mult)

                c3a = wk.tile([128, LRH], fp16, tag='c3a')
                nc.vector.tensor_tensor(
                    c3a[:],
                    _bc(c3p[:, :], [[4 * WH, L], [H, W], [1, H]]),
                    _bc(_off(c3p[:, :], WH), [[4 * WH, L], [H, W], [1, H]]), OP.add)
                c3b = wk.tile([128, LRH], fp16, tag='c3b')
                nc.vector.tensor_tensor(
                    c3b[:],
                    _bc(_off(c3p[:, :], 2 * WH), [[4 * WH, L], [H, W], [1, H]]),
                    _bc(_off(c3p[:, :], 3 * WH), [[4 * WH, L], [H, W], [1, H]]), OP.add)
                c3 = car.tile([128, LRH], fp16, tag='c3')
                nc.vector.tensor_tensor(c3[:], c3a[:], c3b[:], OP.add)

                s['wri'] = wri
                s['c3'] = c3

            # ---- G: issue gathers for tile i ----
            def G(i):
                s = st[i]
                gws = []
                for lv in range(L):
                    gw = gat.tile([128, WIN], bf16, tag=f'gw{lv}', bufs=4)
                    nc.gpsimd.indirect_dma_start(
                        out=gw[:], out_offset=None, in_=vwin,
                        in_offset=bass.IndirectOffsetOnAxis(ap=s['wri'][:, lv:lv + 1], axis=0))
                    gws.append(gw)
                s['gws'] = gws

            # ---- S1: sampling reduce + out-projection issue ----
            def S1(i):
                s = st[i]
                c3 = s['c3']
                prod = wk.tile([128, 4 * WIN], bf16, tag='prod', bufs=2)
                for lv in range(L):
                    nc.vector.tensor_tensor(
                        prod[:, lv * WIN:(lv + 1) * WIN],
                        s['gws'][lv][:],
                        _bc(_off(c3[:, :], lv * WH), [[H, W], [0, 32], [1, H]]),
                        OP.mult)
                s1 = wk.tile([128, 2 * WIN], bf16, tag='s1', bufs=2)
                nc.vector.tensor_tensor(s1[:], prod[:, 0:2 * WIN], prod[:, 2 * WIN:4 * WIN], OP.add)
                s2 = wk.tile([128, WIN], bf16, tag='s2', bufs=2)
                nc.vector.tensor_tensor(s2[:], s1[:, 0:WIN], s1[:, WIN:2 * WIN], OP.add)
                s3 = wk.tile([128, 3 * D], bf16, tag='s3')
                nc.vector.tensor_tensor(s3[:], s2[:, 0:3 * D], s2[:, 3 * D:6 * D], OP.add)
                a1 = wk.tile([128, D], bf16, tag='a1')
                nc.vector.tensor_tensor(a1[:], s3[:, 0:D], s3[:, D:2 * D], OP.add)
                att = wk.tile([128, D], bf16, tag='att')
                nc.vector.tensor_tensor(att[:], a1[:], s3[:, 2 * D:3 * D], OP.add)

                attT = wk.tile([128, D], bf16, tag='attT')
                for kc in range(2):
                    tp = ptr.tile([128, 128], bf16, tag='tr')
                    nc.tensor.transpose(tp[:], att[:, kc * 128:(kc + 1) * 128], identS[:])
                    nc.scalar.activation(attT[:, kc * 128:(kc + 1) * 128], tp[:], AF.Copy)
                s2ps = psB.tile([128, D], f32, tag='p512')
                nc.tensor.matmul(s2ps[:], attT[:, 0:128], woutS[:, 0:D], start=True, stop=False)
                nc.tensor.matmul(s2ps[:], attT[:, 128:256], woutS[:, D:2 * D], start=False, stop=False)
                nc.tensor.matmul(s2ps[:], onesS[:1, :], boutS[:1, :], start=False, stop=True)
                s['s2ps'] = s2ps

            # ---- S2: LN1 + FFN issue ----
            def S2(i):
                rq = i * 128
                s = st[i]
                srco = io.tile([128, D], f32, tag='srco2')
                nc.sync.dma_start(srco[:], srcq[rq:rq + 128, :])
                tet = io.tile([128, D], f32, tag='tet')
                nc.sync.dma_start(tet[:], te_q[rq:rq + 128, :])

                sfull = wk.tile([128, D], f32, tag='sfull')
                nc.vector.tensor_tensor(sfull[:], srco[:], s['s2ps'][:], OP.add)
                bns = wk.tile([128, 6], f32, tag='bns')
                nc.vector.bn_stats(bns[:], sfull[:])
                mv = wk.tile([128, 2], f32, tag='mv')
                nc.vector.bn_aggr(mv[:], bns[:])
                sd = wk.tile([128, 1], f32, tag='sd')
                nc.scalar.activation(sd[:], mv[:, 1:2], AF.Sqrt, bias=epsS[:, 0:1])
                rsd = wk.tile([128, 1], f32, tag='rsd')
                nc.vector.reciprocal(rsd[:], sd[:])
                nmr = wk.tile([128, 1], f32, tag='nmr')
                nc.vector.scalar_tensor_tensor(nmr[:], mv[:, 0:1], -1.0, rsd[:], OP.mult, OP.mult)
                xn = wk.tile([128, D], bf16, tag='xn')
                nc.scalar.activation(xn[:], sfull[:], AF.Identity, scale=rsd[:, 0:1], bias=nmr[:, 0:1])
                teb = wk.tile([128, D], f32, tag='teb')
                nc.gpsimd.tensor_tensor(teb[:], tet[:], ln1bS[:, :], OP.add)
                t1 = wk.tile([128, D], bf16, tag='t1')
                nc.gpsimd.tensor_tensor(t1[:], xn[:], ln1gS[:, :], OP.mult)
                xx = wk.tile([128, D], bf16, tag='xx', bufs=5)
                nc.vector.tensor_tensor(xx[:], t1[:], teb[:], OP.add)

                xT = wk.tile([128, D], bf16, tag='xT')
                for kc in range(2):
                    tp = ptr.tile([128, 128], bf16, tag='tr')
                    nc.tensor.transpose(tp[:], xx[:, kc * 128:(kc + 1) * 128], identS[:])
                    nc.scalar.activation(xT[:, kc * 128:(kc + 1) * 128], tp[:], AF.Copy)
                hbT = wk.tile([128, DF], bf16, tag='hbT', bufs=2)
                for k in range(8):
                    hps = ptr.tile([128, 128], f32, tag='tr')
                    for kc in range(2):
                        nc.tensor.matmul(hps[:], w1S[:, kc * DF + k * 128: kc * DF + (k + 1) * 128],
                                         xT[:, kc * 128:(kc + 1) * 128],
                                         start=(kc == 0), stop=(kc == 1))
                    nc.scalar.activation(hbT[:, k * 128:(k + 1) * 128], hps[:], AF.Relu,
                                         bias=b1tS[:, k:k + 1])
                o2ps = psA.tile([128, D], f32, tag='oaps')
                for k in range(8):
                    nc.tensor.matmul(o2ps[:], hbT[:, k * 128:(k + 1) * 128],
                                     w2S[:, k * D:(k + 1) * D],
                                     start=(k == 0), stop=False)
                nc.tensor.matmul(o2ps[:], onesS[:1, :], b2S[:1, :], start=False, stop=True)
                s['xx'] = xx
                s['o2ps'] = o2ps

            # ---- S3: LN2 + store ----
            def S3(i):
                rq = i * 128
                s = st[i]
                sf2 = wk.tile([128, D], f32, tag='sf2')
                nc.vector.tensor_tensor(sf2[:], s['xx'][:], s['o2ps'][:], OP.add)
                bns2 = wk.tile([128, 6], f32, tag='bns2')
                nc.vector.bn_stats(bns2[:], sf2[:])
                mv2 = wk.tile([128, 2], f32, tag='mv2')
                nc.vector.bn_aggr(mv2[:], bns2[:])
                sd2 = wk.tile([128, 1], f32, tag='sd2')
                nc.scalar.activation(sd2[:], mv2[:, 1:2], AF.Sqrt, bias=epsS[:, 0:1])
                rsd2 = wk.tile([128, 1], f32, tag='rsd2')
                nc.vector.reciprocal(rsd2[:], sd2[:])
                nmr2 = wk.tile([128, 1], f32, tag='nmr2')
                nc.vector.scalar_tensor_tensor(nmr2[:], mv2[:, 0:1], -1.0, rsd2[:], OP.mult, OP.mult)
                xn2 = wk.tile([128, D], bf16, tag='xn2')
                nc.scalar.activation(xn2[:], sf2[:], AF.Identity, scale=rsd2[:, 0:1], bias=nmr2[:, 0:1])
                t2 = wk.tile([128, D], f32, tag='t2')
                nc.gpsimd.tensor_tensor(t2[:], xn2[:], ln2gS[:, :], OP.mult)
                of = wk.tile([128, D], f32, tag='of')
                nc.gpsimd.tensor_tensor(of[:], t2[:], ln2bS[:, :], OP.add)
                nc.sync.dma_start(out_q[rq:rq + 128, :], of[:])

            # ---- emission schedule ----
            NQUAD = LEN_IN // 512  # 30
            n_tiles = int(os.environ.get('K_NTQ', NT_Q))
            for j in range(NQUAD):
                phaseA_quad(j)
                if j < n_tiles:
                    S0(j)
            for i in range(NQUAD, n_tiles):
                S0(i)

            G(0)
            for i in range(n_tiles):
                if i + 1 < n_tiles:
                    G(i + 1)
                S1(i)
                if i >= 2:
                    S3(i - 2)
                S2(i)
            if n_tiles >= 2:
                S3(n_tiles - 2)
            S3(n_tiles - 1)

    if os.environ.get('K_NOSPLIT', '0') != '1':
        _split_excess_waits(nc)
    return nc


_PROG = None
LAST_RESULTS = None


def _get_program():
    global _PROG
    if _PROG is None:
        _PROG = _build_program()
    return _PROG


# host-side layout permutations
_PERM_DH = np.array([h * 32 + d for d in range(32) for h in range(H)])      # value dims
_PERM_LPH = np.array([h * 16 + l * 4 + p                                    # off/attn cols
                      for l in range(L) for p in range(NP) for h in range(H)])


def _host_consts():
    bfc = lambda a: np.ascontiguousarray(np.asarray(a, np.float32)).astype(ml_dtypes.bfloat16)
    c = {}
    c['ident'] = bfc(np.eye(128, dtype=np.float32))
    c['ones1'] = bfc(np.ones((1, 128), np.float32))
    ri = np.zeros((HLP * W,), np.float32)
    for lvp in range(16):
        for r in range(W):
            for h in range(H):
                ri[lvp * 48 + r * 8 + h] = r
    c['riota'] = np.repeat(ri[None, :], 128, axis=0).astype(np.float16)
    c['tsc'] = np.repeat(np.array([LEVEL_LENS], np.float32), 128, axis=0)
    c['slc'] = np.repeat(np.array([[PSTARTS[lv] - 18 for lv in range(L)]], np.int32),
                         128, axis=0)
    return c


def kernel(src, pos, time_embed, reference_points, w_off, b_off, w_attn, b_attn,
           w_val, b_val, w_out, b_out, ln1_g, ln1_b, w1, b1, w2, b2, ln2_g, ln2_b,
           spatial_shapes, level_start_index):
    src = np.asarray(src, np.float32)
    pos = np.asarray(pos, np.float32)
    te = np.asarray(time_embed, np.float32)
    ref = np.asarray(reference_points, np.float32).reshape(2, LEN_IN, L)

    bfc = lambda a: np.ascontiguousarray(np.asarray(a, np.float32)).astype(ml_dtypes.bfloat16)
    consts = _host_consts()

    def chunk2(w):  # [256, X] -> [128, 2X]
        w = np.asarray(w, np.float32)
        return np.concatenate([w[0:128, :], w[128:256, :]], axis=1)

    woa_full = np.concatenate([np.asarray(w_off, np.float32)[:, _PERM_LPH],
                               np.asarray(w_attn, np.float32)[:, _PERM_LPH]], axis=1)
    boa_full = np.concatenate([np.asarray(b_off, np.float32)[_PERM_LPH],
                               np.asarray(b_attn, np.float32)[_PERM_LPH]])[None, :]

    wval_p = np.asarray(w_val, np.float32)[:, _PERM_DH]
    bval_p = np.asarray(b_val, np.float32)[_PERM_DH][None, :]
    bval_p2 = np.concatenate([bval_p, bval_p], axis=1)  # [1, 512] for N=512 bias mm
    wout_p = np.asarray(w_out, np.float32)[_PERM_DH, :]

    w2f = np.asarray(w2, np.float32)
    w2c = np.concatenate([w2f[k * 128:(k + 1) * 128, :] for k in range(8)], axis=1)
    b1f = np.asarray(b1, np.float32)
    b1t = np.stack([b1f[k * 128:(k + 1) * 128] for k in range(8)], axis=1)  # [128, 8]
    rep = lambda v: np.repeat(np.asarray(v, np.float32)[None, :], 128, axis=0)

    shared = {
        'wval': bfc(chunk2(wval_p)),
        'bval': bfc(bval_p2),
        'woa': bfc(chunk2(woa_full)),
        'boa': bfc(boa_full),
        'wout': bfc(chunk2(wout_p)),
        'bout': bfc(np.asarray(b_out, np.float32)[None, :]),
        'w1': bfc(chunk2(np.asarray(w1, np.float32))),
        'b1t': b1t.astype(np.float32),
        'w2': bfc(w2c),
        'b2': bfc(np.asarray(b2, np.float32)[None, :]),
        'ln1g': rep(ln1_g).astype(ml_dtypes.bfloat16), 'ln1b': rep(ln1_b),
        'ln2g': rep(ln2_g).astype(ml_dtypes.bfloat16), 'ln2b': rep(ln2_b),
        **consts,
    }

    in_maps = []
    for c in range(N_CORES):
        n, qr = c // 4, c % 4
        m = dict(shared)
        m['srcT'] = np.ascontiguousarray(src[n].T).astype(ml_dtypes.bfloat16)
        m['srcq'] = src[n, qr * QC:(qr + 1) * QC]
        m['pos_q'] = pos[n, qr * QC:(qr + 1) * QC]
        m['te_q'] = te[n, qr * QC:(qr + 1) * QC]
        m['ref_q'] = ref[n, qr * QC:(qr + 1) * QC]
        in_maps.append(m)

    nc = _get_program()
    from concourse.bass_utils import run_bass_kernel_spmd
    res = run_bass_kernel_spmd(nc, in_maps, core_ids=list(range(N_CORES)))
    global LAST_RESULTS
    LAST_RESULTS = res
    if getattr(res, 'exec_time_ns', None):
        print('HW exec time:', res.exec_time_ns, 'ns')

    out = np.zeros((2, LEN_IN, D), np.float32)
    for c in range(N_CORES):
        n, qr = c // 4, c % 4
        out[n, qr * QC:(qr + 1) * QC] = res.results[c]['out_q']
    return out


# revision 31
# speedup vs baseline: 1.0094x; 1.0094x over previous
"""Deformable transformer encoder layer on 8 Trainium2 NeuronCores.

Sharding: core c handles batch c//4, query-quarter c%4 (3840 queries each).

v3 software-pipelined design:
  - Host-permuted layouts put h innermost (stride 1) everywhere so DVE
    broadcast APs hit the 2x_1p packed mode on the big sampling multiplies;
    hat-weight chain in fp16; value table columns (d,h).
  - Phase A (value projection) uses a host-transposed bf16 srcT (pure
    matmuls, no device transposes), quad 512-row iterations, one load +
    one store DMA each.
  - All 30 query tiles' pre-gather stage (S0: loads, q projection, softmax,
    positions, hat weights) is emitted interleaved with phase A, filling
    the otherwise idle vector/scalar engines during the table build.
  - Main loop is software-pipelined per tile: gathers issued one tile
    ahead; LN2+store of tile i-1 emitted between the sampling stage and
    LN1/FFN of tile i, so the vector engine never waits on the FFN chain.
"""
import os
import sys

sys.path.insert(0, '/opt/trn_rl_repo')

import dataclasses
import numpy as np
import ml_dtypes

import concourse.bass as bass
import concourse.mybir as mybir
from concourse.tile import TileContext

# ---- tile drain workaround (this walrus rejects multi-wait drains) ----
import concourse.tile as _tile_mod
from concourse.tile_sem_assignment import tick_to_sem as _tick_to_sem


def _split_drain_and_barrier(self, tick_clock, wait_clock):
    gc = tick_clock.global_clock
    allocated = self.sems.allocated() if self.sems is not None else {}
    for proc, sem in sorted(allocated.items()):
        t = gc[proc]
        if t > 0:
            self.nc.sync.wait_ge(sem, _tick_to_sem(t, proc))
    self.nc.sync.drain()
    self.nc.all_engine_barrier()
    assert self.sems is not None
    popped = self.nc._tile_sem_poison_stack.pop()
    assert popped is self._sem_poison
    self.nc.clear_and_free_semaphores(list(self.sems.allocated().values()))
    self.nc.all_engine_barrier()


_tile_mod.TileContext._drain_and_barrier = _split_drain_and_barrier

_MAX_WAITS = 1
_wsplit_n = [0]


def _split_excess_waits(nc):
    """Walrus rejects instructions with >2 sem waits; move extras to nops."""
    for f in nc.m.functions:
        for bb in f.blocks:
            out = []
            for inst in list(bb.instructions):
                si = inst.sync_info
                waits = list(si.on_wait) if (si and si.on_wait) else []
                if len(waits) > _MAX_WAITS:
                    extra = waits[:-_MAX_WAITS]
                    keep = waits[-_MAX_WAITS:]
                    for j in range(0, len(extra), _MAX_WAITS):
                        _wsplit_n[0] += 1
                        nop = mybir.InstNoOp(name=f'wsplit-{_wsplit_n[0]}',
                                             ins=[], outs=[])
                        nop.engine = inst.engine
                        nop.sync_info = mybir.SyncInfo(
                            on_wait=extra[j:j + _MAX_WAITS], on_update=[])
                        out.append(nop)
                    inst.sync_info = mybir.SyncInfo(
                        on_wait=keep, on_update=list(si.on_update or []))
                out.append(inst)
            bb.instructions = out

f32 = mybir.dt.float32
bf16 = mybir.dt.bfloat16
fp16 = mybir.dt.float16
i32 = mybir.dt.int32
AF = mybir.ActivationFunctionType
OP = mybir.AluOpType

# ---- problem constants ----
D = 256
DF = 1024
H = 8
L = 4
NP = 4
LEVEL_LENS = (8192, 4096, 2048, 1024)
LEN_IN = 15360
N_CORES = 8
EPS = 1e-5

W = 6
PAD = 8
_starts = []
_acc = PAD
for _t in LEVEL_LENS:
    _starts.append(_acc)
    _acc += _t + PAD
PSTARTS = tuple(_starts)
VROWS = _acc               # 15400
QC = LEN_IN // 4           # 3840
NT_Q = QC // 128           # 30
WIN = W * D                # 1536
HLP = H * L * NP           # 128
WH = W * H                 # 48
LRH = L * W * H            # 192


def _bc(ap, dims):
    """Replace the free dims of a 2-d AP with an explicit dim list."""
    return dataclasses.replace(ap, ap=[list(ap.ap[0])] + [list(d) for d in dims])


def _off(ap, delta):
    return dataclasses.replace(ap, offset=ap.offset + delta)


def _build_program():
    nc = bass.Bass(trn_type='TRN2')

    din = {}
    def I(name, shape, dt):
        din[name] = nc.dram_tensor(name, shape, dt, kind='ExternalInput')
        return din[name]

    srcT = I('srcT', [D, LEN_IN], bf16)      # host-transposed bf16 src
    srcq = I('srcq', [QC, D], f32)
    pos_q = I('pos_q', [QC, D], f32)
    te_q = I('te_q', [QC, D], f32)
    ref_q = I('ref_q', [QC, L], f32)
    wval = I('wval', [128, 2 * D], bf16)     # k-chunk kc at cols [kc*D,(kc+1)*D], out cols (d,h)
    bval = I('bval', [1, 2 * D], bf16)       # (d,h) order, tiled x2
    woa = I('woa', [128, 2 * D], bf16)       # off cols (lv,p,h), attn cols (lv,p,h)
    boa = I('boa', [1, D], bf16)             # concat(b_off, b_attn) permuted
    wout = I('wout', [128, 2 * D], bf16)     # rows (d,h)-permuted
    bout = I('bout', [1, D], bf16)
    w1 = I('w1', [128, 2 * DF], bf16)        # chunk kc at cols [kc*DF,(kc+1)*DF]
    b1t = I('b1t', [128, 8], f32)            # b1 transposed: [dffn%128, dffn//128]
    w2 = I('w2', [128, 8 * D], bf16)         # chunk k at cols [k*D,(k+1)*D]
    b2 = I('b2', [1, D], bf16)
    ln1g = I('ln1g', [128, D], bf16)         # replicated over partitions
    ln1b = I('ln1b', [128, D], f32)
    ln2g = I('ln2g', [128, D], bf16)
    ln2b = I('ln2b', [128, D], f32)
    ident = I('ident', [128, 128], bf16)
    ones1 = I('ones1', [1, 128], bf16)
    riota = I('riota', [128, HLP * W], fp16) # col (lv,p)*48 + r*8 + h -> r
    tsc = I('tsc', [128, L], f32)            # replicated level lens
    slc = I('slc', [128, L], i32)            # replicated PSTARTS[l] - 18

    out_q = nc.dram_tensor('out_q', [QC, D], f32, kind='ExternalOutput')

    with TileContext(nc) as tc:
        with tc.tile_pool(name='cst', bufs=1) as cst, \
             tc.tile_pool(name='io', bufs=3) as io, \
             tc.tile_pool(name='car', bufs=32) as car, \
             tc.tile_pool(name='wk', bufs=3) as wk, \
             tc.tile_pool(name='gat', bufs=3) as gat, \
             tc.tile_pool(name='psA', bufs=3, space='PSUM') as psA, \
             tc.tile_pool(name='psB', bufs=2, space='PSUM') as psB, \
             tc.tile_pool(name='ptr', bufs=3, space='PSUM') as ptr, \
             tc.tile_pool(name='dram', bufs=1, space='DRAM') as dram:

            value_dram = dram.tile([VROWS, D], bf16)

            def ctile(name, dt=bf16):
                t = cst.tile(list(din[name].shape), dt, tag=name)
                nc.sync.dma_start(t[:], din[name][:, :])
                return t

            identS = ctile('ident')
            onesS = ctile('ones1')
            riotaS = ctile('riota', fp16)
            tscS = ctile('tsc', f32)
            slcS = ctile('slc', i32)
            wvalS = ctile('wval')
            bvalS = ctile('bval')
            woaS = ctile('woa')
            boaS = ctile('boa')
            woutS = ctile('wout')
            boutS = ctile('bout')
            w1S = ctile('w1')
            b1tS = ctile('b1t', f32)
            w2S = ctile('w2')
            b2S = ctile('b2')
            ln1gS = ctile('ln1g', bf16)
            ln1bS = ctile('ln1b', f32)
            ln2gS = ctile('ln2g', bf16)
            ln2bS = ctile('ln2b', f32)

            epsS = cst.tile([128, 1], f32, tag='epsS')
            nc.vector.memset(epsS[:], EPS)
            zpad = cst.tile([PAD, D], bf16, tag='zpad')
            nc.vector.memset(zpad[:], 0.0)
            nc.sync.dma_start(value_dram[0:PAD, :], zpad[:])
            for lv in range(L):
                r0 = PSTARTS[lv] + LEVEL_LENS[lv]
                nc.sync.dma_start(value_dram[r0:r0 + PAD, :], zpad[:])

            vwin = dataclasses.replace(value_dram[:, :], ap=[[D, VROWS], [1, D]])
            st = [dict() for _ in range(NT_Q)]

            # ---- phase A quad: 512 value rows ----
            cums = [0]
            for t in LEVEL_LENS:
                cums.append(cums[-1] + t)

            def phaseA_quad(j):
                r = j * 512
                lv = next(k for k in range(L) if r < cums[k + 1])
                prow = PSTARTS[lv] + (r - cums[lv])
                sT = io.tile([128, 1024], bf16, tag='va_in')
                src_ap = dataclasses.replace(
                    srcT[:, :], offset=r,
                    ap=[[LEN_IN, 128], [LEN_IN * 128, 2], [128, 4], [1, 128]])
                nc.sync.dma_start(sT[:], src_ap)
                v_b = wk.tile([128, 1024], bf16, tag='va_out', bufs=2)
                for half in range(2):
                    vps = psB.tile([128, 512], f32, tag='p512')
                    for t4 in range(2):
                        j4 = half * 2 + t4
                        for h in range(2):
                            # start=True on the first matmul zeroes the whole
                            # 2KB zero region (both 256-col halves); everything
                            # else accumulates within the single group.
                            nc.tensor.matmul(
                                vps[:, t4 * 256:(t4 + 1) * 256],
                                sT[:, h * 512 + j4 * 128: h * 512 + (j4 + 1) * 128],
                                wvalS[:, h * D:(h + 1) * D],
                                start=(t4 == 0 and h == 0), stop=False)
                    nc.tensor.matmul(vps[:], onesS[:1, :], bvalS[:1, :],
                                     start=False, stop=True)
                    nc.scalar.activation(v_b[:, half * 512:(half + 1) * 512], vps[:], AF.Copy)
                out_ap = dataclasses.replace(
                    value_dram[:, :], offset=prow * D,
                    ap=[[D, 128], [D * 128, 4], [1, D]])
                nc.sync.dma_start(out_ap, v_b[:])

            # ---- S0: pre-gather stage for tile i ----
            def S0(i):
                rq = i * 128
                s = st[i]
                srco = io.tile([128, D], f32, tag='srco')
                nc.sync.dma_start(srco[:], srcq[rq:rq + 128, :])
                post = io.tile([128, D], f32, tag='post')
                nc.sync.dma_start(post[:], pos_q[rq:rq + 128, :])
                reft = io.tile([128, L], f32, tag='reft')
                nc.sync.dma_start(reft[:], ref_q[rq:rq + 128, :])

                qb = wk.tile([128, D], bf16, tag='qb')
                nc.gpsimd.tensor_tensor(qb[:], srco[:], post[:], OP.add)
                qT = wk.tile([128, D], bf16, tag='qT')
                for kc in range(2):
                    tp = ptr.tile([128, 128], bf16, tag='tr')
                    nc.tensor.transpose(tp[:], qb[:, kc * 128:(kc + 1) * 128], identS[:])
                    nc.scalar.activation(qT[:, kc * 128:(kc + 1) * 128], tp[:], AF.Copy)

                oaps = psA.tile([128, D], f32, tag='oaps')
                nc.tensor.matmul(oaps[:], qT[:, 0:128], woaS[:, 0:D], start=True, stop=False)
                nc.tensor.matmul(oaps[:], qT[:, 128:256], woaS[:, D:2 * D], start=False, stop=False)
                nc.tensor.matmul(oaps[:], onesS[:1, :], boaS[:1, :], start=False, stop=True)

                # softmax over (lv,p) per h; cols (lv,p,h), h innermost
                ex = wk.tile([128, 128], f32, tag='ex')
                nc.scalar.activation(ex[:], oaps[:, 128:256], AF.Exp)
                zs = wk.tile([128, 8], f32, tag='zs')
                nc.vector.tensor_reduce(zs[:], _bc(ex[:, :], [[1, H], [H, 16]]),
                                        mybir.AxisListType.X, OP.add)
                zr = wk.tile([128, 8], f32, tag='zr')
                nc.vector.reciprocal(zr[:], zs[:])
                wnh = wk.tile([128, 128], fp16, tag='wnh')
                nc.vector.tensor_tensor(
                    _bc(wnh[:, :], [[H, 16], [1, H]]),
                    _bc(ex[:, :], [[H, 16], [1, H]]),
                    _bc(zr[:, :], [[0, 16], [1, H]]), OP.mult)

                # positions
                art = wk.tile([128, L], f32, tag='art')
                nc.vector.tensor_tensor(art[:], reft[:], tscS[:, :], OP.mult)
                ar16 = wk.tile([128, L], f32, tag='ar16')
                nc.vector.tensor_scalar(ar16[:], art[:], 15.5, None, OP.add)
                fli = wk.tile([128, L], i32, tag='fli')
                nc.vector.tensor_copy(fli[:], ar16[:])
                fl16 = wk.tile([128, L], f32, tag='fl16')
                nc.vector.tensor_copy(fl16[:], fli[:])
                wri = car.tile([128, L], i32, tag='wri')
                nc.vector.tensor_tensor(wri[:], fli[:], slcS[:, :], OP.add)
                arf = wk.tile([128, L], f32, tag='arf')
                nc.vector.scalar_tensor_tensor(arf[:], ar16[:], 2.0, fl16[:], OP.add, OP.subtract)

                xq = wk.tile([128, HLP], fp16, tag='xq')
                nc.vector.tensor_tensor(
                    xq[:], oaps[:, 0:128],
                    _bc(arf[:, :], [[1, L], [0, NP], [0, H]]), OP.add)

                dd = wk.tile([128, HLP * W], fp16, tag='dd', bufs=2)
                nc.vector.tensor_tensor(
                    dd[:], riotaS[:, :],
                    _bc(xq[:, :], [[H, 16], [0, W], [1, H]]), OP.subtract)
                da = wk.tile([128, HLP * W], fp16, tag='da', bufs=2)
                nc.scalar.activation(da[:], dd[:], AF.Abs)
                h1 = wk.tile([128, HLP * W], fp16, tag='h1', bufs=2)
                nc.scalar.activation(h1[:], da[:], AF.Relu, bias=1.0, scale=-1.0)
                c3p = wk.tile([128, HLP * W], fp16, tag='c3p')
                nc.vector.tensor_tensor(
                    c3p[:], h1[:],
                    _bc(wnh[:, :], [[H, 16], [0, W], [1, H]]), OP.mult)

                c3a = wk.tile([128, LRH], fp16, tag='c3a')
                nc.vector.tensor_tensor(
                    c3a[:],
                    _bc(c3p[:, :], [[4 * WH, L], [H, W], [1, H]]),
                    _bc(_off(c3p[:, :], WH), [[4 * WH, L], [H, W], [1, H]]), OP.add)
                c3b = wk.tile([128, LRH], fp16, tag='c3b')
                nc.vector.tensor_tensor(
                    c3b[:],
                    _bc(_off(c3p[:, :], 2 * WH), [[4 * WH, L], [H, W], [1, H]]),
                    _bc(_off(c3p[:, :], 3 * WH), [[4 * WH, L], [H, W], [1, H]]), OP.add)
                c3 = car.tile([128, LRH], fp16, tag='c3')
                nc.vector.tensor_tensor(c3[:], c3a[:], c3b[:], OP.add)

                s['wri'] = wri
                s['c3'] = c3

            # ---- G: issue gathers for tile i ----
            def G(i):
                s = st[i]
                gws = []
                for lv in range(L):
                    gw = gat.tile([128, WIN], bf16, tag=f'gw{lv}', bufs=3)
                    nc.gpsimd.indirect_dma_start(
                        out=gw[:], out_offset=None, in_=vwin,
                        in_offset=bass.IndirectOffsetOnAxis(ap=s['wri'][:, lv:lv + 1], axis=0))
                    gws.append(gw)
                s['gws'] = gws

            # ---- S1: sampling reduce + out-projection issue ----
            def S1(i):
                s = st[i]
                c3 = s['c3']
                prod = wk.tile([128, 4 * WIN], bf16, tag='prod', bufs=2)
                for lv in range(L):
                    nc.vector.tensor_tensor(
                        prod[:, lv * WIN:(lv + 1) * WIN],
                        s['gws'][lv][:],
                        _bc(_off(c3[:, :], lv * WH), [[H, W], [0, 32], [1, H]]),
                        OP.mult)
                s1 = wk.tile([128, 2 * WIN], bf16, tag='s1', bufs=2)
                nc.vector.tensor_tensor(s1[:], prod[:, 0:2 * WIN], prod[:, 2 * WIN:4 * WIN], OP.add)
                s2 = wk.tile([128, WIN], bf16, tag='s2', bufs=2)
                nc.vector.tensor_tensor(s2[:], s1[:, 0:WIN], s1[:, WIN:2 * WIN], OP.add)
                s3 = wk.tile([128, 3 * D], bf16, tag='s3')
                nc.vector.tensor_tensor(s3[:], s2[:, 0:3 * D], s2[:, 3 * D:6 * D], OP.add)
                a1 = wk.tile([128, D], bf16, tag='a1')
                nc.vector.tensor_tensor(a1[:], s3[:, 0:D], s3[:, D:2 * D], OP.add)
                att = wk.tile([128, D], bf16, tag='att')
                nc.vector.tensor_tensor(att[:], a1[:], s3[:, 2 * D:3 * D], OP.add)

                attT = wk.tile([128, D], bf16, tag='attT')
                for kc in range(2):
                    tp = ptr.tile([128, 128], bf16, tag='tr')
                    nc.tensor.transpose(tp[:], att[:, kc * 128:(kc + 1) * 128], identS[:])
                    nc.scalar.activation(attT[:, kc * 128:(kc + 1) * 128], tp[:], AF.Copy)
                s2ps = psB.tile([128, D], f32, tag='p512')
                nc.tensor.matmul(s2ps[:], attT[:, 0:128], woutS[:, 0:D], start=True, stop=False)
                nc.tensor.matmul(s2ps[:], attT[:, 128:256], woutS[:, D:2 * D], start=False, stop=False)
                nc.tensor.matmul(s2ps[:], onesS[:1, :], boutS[:1, :], start=False, stop=True)
                s['s2ps'] = s2ps

            # ---- S2: LN1 + FFN issue ----
            def S2(i):
                rq = i * 128
                s = st[i]
                srco = io.tile([128, D], f32, tag='srco2')
                nc.sync.dma_start(srco[:], srcq[rq:rq + 128, :])
                tet = io.tile([128, D], f32, tag='tet')
                nc.sync.dma_start(tet[:], te_q[rq:rq + 128, :])

                sfull = wk.tile([128, D], f32, tag='sfull')
                nc.vector.tensor_tensor(sfull[:], srco[:], s['s2ps'][:], OP.add)
                bns = wk.tile([128, 6], f32, tag='bns')
                nc.vector.bn_stats(bns[:], sfull[:])
                mv = wk.tile([128, 2], f32, tag='mv')
                nc.vector.bn_aggr(mv[:], bns[:])
                sd = wk.tile([128, 1], f32, tag='sd')
                nc.scalar.activation(sd[:], mv[:, 1:2], AF.Sqrt, bias=epsS[:, 0:1])
                rsd = wk.tile([128, 1], f32, tag='rsd')
                nc.vector.reciprocal(rsd[:], sd[:])
                nmr = wk.tile([128, 1], f32, tag='nmr')
                nc.vector.scalar_tensor_tensor(nmr[:], mv[:, 0:1], -1.0, rsd[:], OP.mult, OP.mult)
                xn = wk.tile([128, D], bf16, tag='xn')
                nc.scalar.activation(xn[:], sfull[:], AF.Identity, scale=rsd[:, 0:1], bias=nmr[:, 0:1])
                teb = wk.tile([128, D], f32, tag='teb')
                nc.gpsimd.tensor_tensor(teb[:], tet[:], ln1bS[:, :], OP.add)
                t1 = wk.tile([128, D], bf16, tag='t1')
                nc.gpsimd.tensor_tensor(t1[:], xn[:], ln1gS[:, :], OP.mult)
                xx = wk.tile([128, D], bf16, tag='xx', bufs=5)
                nc.vector.tensor_tensor(xx[:], t1[:], teb[:], OP.add)

                xT = wk.tile([128, D], bf16, tag='xT')
                for kc in range(2):
                    tp = ptr.tile([128, 128], bf16, tag='tr')
                    nc.tensor.transpose(tp[:], xx[:, kc * 128:(kc + 1) * 128], identS[:])
                    nc.scalar.activation(xT[:, kc * 128:(kc + 1) * 128], tp[:], AF.Copy)
                hbT = wk.tile([128, DF], bf16, tag='hbT', bufs=2)
                for k in range(8):
                    hps = ptr.tile([128, 128], f32, tag='tr')
                    for kc in range(2):
                        nc.tensor.matmul(hps[:], w1S[:, kc * DF + k * 128: kc * DF + (k + 1) * 128],
                                         xT[:, kc * 128:(kc + 1) * 128],
                                         start=(kc == 0), stop=(kc == 1))
                    nc.scalar.activation(hbT[:, k * 128:(k + 1) * 128], hps[:], AF.Relu,
                                         bias=b1tS[:, k:k + 1])
                o2ps = psA.tile([128, D], f32, tag='oaps')
                for k in range(8):
                    nc.tensor.matmul(o2ps[:], hbT[:, k * 128:(k + 1) * 128],
                                     w2S[:, k * D:(k + 1) * D],
                                     start=(k == 0), stop=False)
                nc.tensor.matmul(o2ps[:], onesS[:1, :], b2S[:1, :], start=False, stop=True)
                s['xx'] = xx
                s['o2ps'] = o2ps

            # ---- S3: LN2 + store ----
            def S3(i):
                rq = i * 128
                s = st[i]
                sf2 = wk.tile([128, D], f32, tag='sf2')
                nc.vector.tensor_tensor(sf2[:], s['xx'][:], s['o2ps'][:], OP.add)
                bns2 = wk.tile([128, 6], f32, tag='bns2')
                nc.vector.bn_stats(bns2[:], sf2[:])
                mv2 = wk.tile([128, 2], f32, tag='mv2')
                nc.vector.bn_aggr(mv2[:], bns2[:])
                sd2 = wk.tile([128, 1], f32, tag='sd2')
                nc.scalar.activation(sd2[:], mv2[:, 1:2], AF.Sqrt, bias=epsS[:, 0:1])
                rsd2 = wk.tile([128, 1], f32, tag='rsd2')
                nc.vector.reciprocal(rsd2[:], sd2[:])
                nmr2 = wk.tile([128, 1], f32, tag='nmr2')
                nc.vector.scalar_tensor_tensor(nmr2[:], mv2[:, 0:1], -1.0, rsd2[:], OP.mult, OP.mult)
                xn2 = wk.tile([128, D], bf16, tag='xn2')
                nc.scalar.activation(xn2[:], sf2[:], AF.Identity, scale=rsd2[:, 0:1], bias=nmr2[:, 0:1])
                t2 = wk.tile([128, D], f32, tag='t2')
                nc.gpsimd.tensor_tensor(t2[:], xn2[:], ln2gS[:, :], OP.mult)
                of = wk.tile([128, D], f32, tag='of')
                nc.gpsimd.tensor_tensor(of[:], t2[:], ln2bS[:, :], OP.add)
                nc.sync.dma_start(out_q[rq:rq + 128, :], of[:])

            # ---- emission schedule ----
            NQUAD = LEN_IN // 512  # 30
            n_tiles = int(os.environ.get('K_NTQ', NT_Q))
            for j in range(NQUAD):
                phaseA_quad(j)
                if j < n_tiles:
                    S0(j)
            for i in range(NQUAD, n_tiles):
                S0(i)

            G(0)
            for i in range(n_tiles):
                if i + 1 < n_tiles:
                    G(i + 1)
                S1(i)
                if i >= 2:
                    S3(i - 2)
                S2(i)
            if n_tiles >= 2:
                S3(n_tiles - 2)
            S3(n_tiles - 1)

    if os.environ.get('K_NOSPLIT', '0') != '1':
        _split_excess_waits(nc)
    return nc


_PROG = None
LAST_RESULTS = None


def _get_program():
    global _PROG
    if _PROG is None:
        _PROG = _build_program()
    return _PROG


# host-side layout permutations
_PERM_DH = np.array([h * 32 + d for d in range(32) for h in range(H)])      # value dims
_PERM_LPH = np.array([h * 16 + l * 4 + p                                    # off/attn cols
                      for l in range(L) for p in range(NP) for h in range(H)])


def _host_consts():
    bfc = lambda a: np.ascontiguousarray(np.asarray(a, np.float32)).astype(ml_dtypes.bfloat16)
    c = {}
    c['ident'] = bfc(np.eye(128, dtype=np.float32))
    c['ones1'] = bfc(np.ones((1, 128), np.float32))
    ri = np.zeros((HLP * W,), np.float32)
    for lvp in range(16):
        for r in range(W):
            for h in range(H):
                ri[lvp * 48 + r * 8 + h] = r
    c['riota'] = np.repeat(ri[None, :], 128, axis=0).astype(np.float16)
    c['tsc'] = np.repeat(np.array([LEVEL_LENS], np.float32), 128, axis=0)
    c['slc'] = np.repeat(np.array([[PSTARTS[lv] - 18 for lv in range(L)]], np.int32),
                         128, axis=0)
    return c


def kernel(src, pos, time_embed, reference_points, w_off, b_off, w_attn, b_attn,
           w_val, b_val, w_out, b_out, ln1_g, ln1_b, w1, b1, w2, b2, ln2_g, ln2_b,
           spatial_shapes, level_start_index):
    src = np.asarray(src, np.float32)
    pos = np.asarray(pos, np.float32)
    te = np.asarray(time_embed, np.float32)
    ref = np.asarray(reference_points, np.float32).reshape(2, LEN_IN, L)

    bfc = lambda a: np.ascontiguousarray(np.asarray(a, np.float32)).astype(ml_dtypes.bfloat16)
    consts = _host_consts()

    def chunk2(w):  # [256, X] -> [128, 2X]
        w = np.asarray(w, np.float32)
        return np.concatenate([w[0:128, :], w[128:256, :]], axis=1)

    woa_full = np.concatenate([np.asarray(w_off, np.float32)[:, _PERM_LPH],
                               np.asarray(w_attn, np.float32)[:, _PERM_LPH]], axis=1)
    boa_full = np.concatenate([np.asarray(b_off, np.float32)[_PERM_LPH],
                               np.asarray(b_attn, np.float32)[_PERM_LPH]])[None, :]

    wval_p = np.asarray(w_val, np.float32)[:, _PERM_DH]
    bval_p = np.asarray(b_val, np.float32)[_PERM_DH][None, :]
    bval_p2 = np.concatenate([bval_p, bval_p], axis=1)  # [1, 512] for N=512 bias mm
    wout_p = np.asarray(w_out, np.float32)[_PERM_DH, :]

    w2f = np.asarray(w2, np.float32)
    w2c = np.concatenate([w2f[k * 128:(k + 1) * 128, :] for k in range(8)], axis=1)
    b1f = np.asarray(b1, np.float32)
    b1t = np.stack([b1f[k * 128:(k + 1) * 128] for k in range(8)], axis=1)  # [128, 8]
    rep = lambda v: np.repeat(np.asarray(v, np.float32)[None, :], 128, axis=0)

    shared = {
        'wval': bfc(chunk2(wval_p)),
        'bval': bfc(bval_p2),
        'woa': bfc(chunk2(woa_full)),
        'boa': bfc(boa_full),
        'wout': bfc(chunk2(wout_p)),
        'bout': bfc(np.asarray(b_out, np.float32)[None, :]),
        'w1': bfc(chunk2(np.asarray(w1, np.float32))),
        'b1t': b1t.astype(np.float32),
        'w2': bfc(w2c),
        'b2': bfc(np.asarray(b2, np.float32)[None, :]),
        'ln1g': rep(ln1_g).astype(ml_dtypes.bfloat16), 'ln1b': rep(ln1_b),
        'ln2g': rep(ln2_g).astype(ml_dtypes.bfloat16), 'ln2b': rep(ln2_b),
        **consts,
    }

    in_maps = []
    for c in range(N_CORES):
        n, qr = c // 4, c % 4
        m = dict(shared)
        m['srcT'] = np.ascontiguousarray(src[n].T).astype(ml_dtypes.bfloat16)
        m['srcq'] = src[n, qr * QC:(qr + 1) * QC]
        m['pos_q'] = pos[n, qr * QC:(qr + 1) * QC]
        m['te_q'] = te[n, qr * QC:(qr + 1) * QC]
        m['ref_q'] = ref[n, qr * QC:(qr + 1) * QC]
        in_maps.append(m)

    nc = _get_program()
    from concourse.bass_utils import run_bass_kernel_spmd
    res = run_bass_kernel_spmd(nc, in_maps, core_ids=list(range(N_CORES)))
    global LAST_RESULTS
    LAST_RESULTS = res
    if getattr(res, 'exec_time_ns', None):
        print('HW exec time:', res.exec_time_ns, 'ns')

    out = np.zeros((2, LEN_IN, D), np.float32)
    for c in range(N_CORES):
        n, qr = c // 4, c % 4
        out[n, qr * QC:(qr + 1) * QC] = res.results[c]['out_q']
    return out


# revision 33
# speedup vs baseline: 1.0118x; 1.0024x over previous
"""Deformable transformer encoder layer on 8 Trainium2 NeuronCores.

Sharding: core c handles batch c//4, query-quarter c%4 (3840 queries each).

Software-pipelined design (final):
  - Host-permuted layouts put h innermost (stride 1) everywhere so DVE
    broadcast APs hit the 2x_1p packed mode on the big sampling multiplies;
    hat-weight chain in fp16; value table columns (d,h).
  - Phase A (value projection) uses a host-transposed bf16 srcT (pure
    matmuls, no device transposes), quad 512-row iterations, one load +
    one store DMA each.
  - All 30 query tiles' pre-gather stage (S0: loads, q projection, softmax,
    positions, hat weights) is emitted interleaved with phase A, filling
    the otherwise idle vector/scalar engines during the table build.
  - Main loop is software-pipelined per tile: gathers issued one tile
    ahead; LN2+store of tile i-2 emitted between the sampling stage and
    LN1/FFN of tile i, so the vector engine never waits on the FFN chain.
  - Residual adds / LN gain-bias applies that sit off the vector critical
    path run on gpsimd; LayerNorm stats via bn_stats/bn_aggr; biases
    folded into K=1 ones-row matmuls.
"""
import os
import sys

sys.path.insert(0, '/opt/trn_rl_repo')

import dataclasses
import numpy as np
import ml_dtypes

import concourse.bass as bass
import concourse.mybir as mybir
from concourse.tile import TileContext

# ---- tile drain workaround (this walrus rejects multi-wait drains) ----
import concourse.tile as _tile_mod
from concourse.tile_sem_assignment import tick_to_sem as _tick_to_sem


def _split_drain_and_barrier(self, tick_clock, wait_clock):
    gc = tick_clock.global_clock
    allocated = self.sems.allocated() if self.sems is not None else {}
    for proc, sem in sorted(allocated.items()):
        t = gc[proc]
        if t > 0:
            self.nc.sync.wait_ge(sem, _tick_to_sem(t, proc))
    self.nc.sync.drain()
    self.nc.all_engine_barrier()
    assert self.sems is not None
    popped = self.nc._tile_sem_poison_stack.pop()
    assert popped is self._sem_poison
    self.nc.clear_and_free_semaphores(list(self.sems.allocated().values()))
    self.nc.all_engine_barrier()


_tile_mod.TileContext._drain_and_barrier = _split_drain_and_barrier

_MAX_WAITS = 1
_wsplit_n = [0]


def _split_excess_waits(nc):
    """Walrus rejects instructions with >2 sem waits; move extras to nops."""
    for f in nc.m.functions:
        for bb in f.blocks:
            out = []
            for inst in list(bb.instructions):
                si = inst.sync_info
                waits = list(si.on_wait) if (si and si.on_wait) else []
                if len(waits) > _MAX_WAITS:
                    extra = waits[:-_MAX_WAITS]
                    keep = waits[-_MAX_WAITS:]
                    for j in range(0, len(extra), _MAX_WAITS):
                        _wsplit_n[0] += 1
                        nop = mybir.InstNoOp(name=f'wsplit-{_wsplit_n[0]}',
                                             ins=[], outs=[])
                        nop.engine = inst.engine
                        nop.sync_info = mybir.SyncInfo(
                            on_wait=extra[j:j + _MAX_WAITS], on_update=[])
                        out.append(nop)
                    inst.sync_info = mybir.SyncInfo(
                        on_wait=keep, on_update=list(si.on_update or []))
                out.append(inst)
            bb.instructions = out

f32 = mybir.dt.float32
bf16 = mybir.dt.bfloat16
fp16 = mybir.dt.float16
i32 = mybir.dt.int32
AF = mybir.ActivationFunctionType
OP = mybir.AluOpType

# ---- problem constants ----
D = 256
DF = 1024
H = 8
L = 4
NP = 4
LEVEL_LENS = (8192, 4096, 2048, 1024)
LEN_IN = 15360
N_CORES = 8
EPS = 1e-5

W = 6
PAD = 8
_starts = []
_acc = PAD
for _t in LEVEL_LENS:
    _starts.append(_acc)
    _acc += _t + PAD
PSTARTS = tuple(_starts)
VROWS = _acc               # 15400
QC = LEN_IN // 4           # 3840
NT_Q = QC // 128           # 30
WIN = W * D                # 1536
HLP = H * L * NP           # 128
WH = W * H                 # 48
LRH = L * W * H            # 192


def _bc(ap, dims):
    """Replace the free dims of a 2-d AP with an explicit dim list."""
    return dataclasses.replace(ap, ap=[list(ap.ap[0])] + [list(d) for d in dims])


def _off(ap, delta):
    return dataclasses.replace(ap, offset=ap.offset + delta)


def _build_program():
    nc = bass.Bass(trn_type='TRN2')

    din = {}
    def I(name, shape, dt):
        din[name] = nc.dram_tensor(name, shape, dt, kind='ExternalInput')
        return din[name]

    srcT = I('srcT', [D, LEN_IN], bf16)      # host-transposed bf16 src
    srcq = I('srcq', [QC, D], f32)
    pos_q = I('pos_q', [QC, D], f32)
    te_q = I('te_q', [QC, D], f32)
    ref_q = I('ref_q', [QC, L], f32)
    wval = I('wval', [128, 2 * D], bf16)     # k-chunk kc at cols [kc*D,(kc+1)*D], out cols (d,h)
    bval = I('bval', [1, 2 * D], bf16)       # (d,h) order, tiled x2
    woa = I('woa', [128, 2 * D], bf16)       # off cols (lv,p,h), attn cols (lv,p,h)
    boa = I('boa', [1, D], bf16)             # concat(b_off, b_attn) permuted
    wout = I('wout', [128, 2 * D], bf16)     # rows (d,h)-permuted
    bout = I('bout', [1, D], bf16)
    w1 = I('w1', [128, 2 * DF], bf16)        # chunk kc at cols [kc*DF,(kc+1)*DF]
    b1t = I('b1t', [128, 8], f32)            # b1 transposed: [dffn%128, dffn//128]
    w2 = I('w2', [128, 8 * D], bf16)         # chunk k at cols [k*D,(k+1)*D]
    b2 = I('b2', [1, D], bf16)
    ln1g = I('ln1g', [128, D], bf16)         # replicated over partitions
    ln1b = I('ln1b', [128, D], f32)
    ln2g = I('ln2g', [128, D], bf16)
    ln2b = I('ln2b', [128, D], f32)
    ident = I('ident', [128, 128], bf16)
    ones1 = I('ones1', [1, 128], bf16)
    riota = I('riota', [128, HLP * W], fp16) # col (lv,p)*48 + r*8 + h -> r
    tsc = I('tsc', [128, L], f32)            # replicated level lens
    slc = I('slc', [128, L], i32)            # replicated PSTARTS[l] - 18

    out_q = nc.dram_tensor('out_q', [QC, D], f32, kind='ExternalOutput')

    with TileContext(nc) as tc:
        with tc.tile_pool(name='cst', bufs=1) as cst, \
             tc.tile_pool(name='io', bufs=3) as io, \
             tc.tile_pool(name='car', bufs=32) as car, \
             tc.tile_pool(name='wk', bufs=3) as wk, \
             tc.tile_pool(name='gat', bufs=3) as gat, \
             tc.tile_pool(name='psA', bufs=3, space='PSUM') as psA, \
             tc.tile_pool(name='psB', bufs=2, space='PSUM') as psB, \
             tc.tile_pool(name='ptr', bufs=3, space='PSUM') as ptr, \
             tc.tile_pool(name='dram', bufs=1, space='DRAM') as dram:

            value_dram = dram.tile([VROWS, D], bf16)

            def ctile(name, dt=bf16):
                t = cst.tile(list(din[name].shape), dt, tag=name)
                nc.sync.dma_start(t[:], din[name][:, :])
                return t

            identS = ctile('ident')
            onesS = ctile('ones1')
            riotaS = ctile('riota', fp16)
            tscS = ctile('tsc', f32)
            slcS = ctile('slc', i32)
            wvalS = ctile('wval')
            bvalS = ctile('bval')
            woaS = ctile('woa')
            boaS = ctile('boa')
            woutS = ctile('wout')
            boutS = ctile('bout')
            w1S = ctile('w1')
            b1tS = ctile('b1t', f32)
            w2S = ctile('w2')
            b2S = ctile('b2')
            ln1gS = ctile('ln1g', bf16)
            ln1bS = ctile('ln1b', f32)
            ln2gS = ctile('ln2g', bf16)
            ln2bS = ctile('ln2b', f32)

            epsS = cst.tile([128, 1], f32, tag='epsS')
            nc.vector.memset(epsS[:], EPS)
            zpad = cst.tile([PAD, D], bf16, tag='zpad')
            nc.vector.memset(zpad[:], 0.0)
            nc.sync.dma_start(value_dram[0:PAD, :], zpad[:])
            for lv in range(L):
                r0 = PSTARTS[lv] + LEVEL_LENS[lv]
                nc.sync.dma_start(value_dram[r0:r0 + PAD, :], zpad[:])

            vwin = dataclasses.replace(value_dram[:, :], ap=[[D, VROWS], [1, D]])
            st = [dict() for _ in range(NT_Q)]

            # ---- phase A quad: 512 value rows ----
            cums = [0]
            for t in LEVEL_LENS:
                cums.append(cums[-1] + t)

            def phaseA_quad(j):
                r = j * 512
                lv = next(k for k in range(L) if r < cums[k + 1])
                prow = PSTARTS[lv] + (r - cums[lv])
                sT = io.tile([128, 1024], bf16, tag='va_in')
                src_ap = dataclasses.replace(
                    srcT[:, :], offset=r,
                    ap=[[LEN_IN, 128], [LEN_IN * 128, 2], [128, 4], [1, 128]])
                nc.sync.dma_start(sT[:], src_ap)
                v_b = wk.tile([128, 1024], bf16, tag='va_out', bufs=2)
                for half in range(2):
                    vps = psB.tile([128, 512], f32, tag='p512')
                    for t4 in range(2):
                        j4 = half * 2 + t4
                        for h in range(2):
                            # start=True on the first matmul zeroes the whole
                            # 2KB zero region (both 256-col halves); everything
                            # else accumulates within the single group.
                            nc.tensor.matmul(
                                vps[:, t4 * 256:(t4 + 1) * 256],
                                sT[:, h * 512 + j4 * 128: h * 512 + (j4 + 1) * 128],
                                wvalS[:, h * D:(h + 1) * D],
                                start=(t4 == 0 and h == 0), stop=False)
                    nc.tensor.matmul(vps[:], onesS[:1, :], bvalS[:1, :],
                                     start=False, stop=True)
                    nc.scalar.activation(v_b[:, half * 512:(half + 1) * 512], vps[:], AF.Copy)
                out_ap = dataclasses.replace(
                    value_dram[:, :], offset=prow * D,
                    ap=[[D, 128], [D * 128, 4], [1, D]])
                nc.sync.dma_start(out_ap, v_b[:])

            # ---- S0: pre-gather stage for tile i ----
            def S0(i):
                rq = i * 128
                s = st[i]
                srco = io.tile([128, D], f32, tag='srco')
                nc.sync.dma_start(srco[:], srcq[rq:rq + 128, :])
                post = io.tile([128, D], f32, tag='post')
                nc.sync.dma_start(post[:], pos_q[rq:rq + 128, :])
                reft = io.tile([128, L], f32, tag='reft')
                nc.sync.dma_start(reft[:], ref_q[rq:rq + 128, :])

                qb = wk.tile([128, D], bf16, tag='qb')
                nc.gpsimd.tensor_tensor(qb[:], srco[:], post[:], OP.add)
                qT = wk.tile([128, D], bf16, tag='qT')
                for kc in range(2):
                    tp = ptr.tile([128, 128], bf16, tag='tr')
                    nc.tensor.transpose(tp[:], qb[:, kc * 128:(kc + 1) * 128], identS[:])
                    nc.scalar.activation(qT[:, kc * 128:(kc + 1) * 128], tp[:], AF.Copy)

                oaps = psA.tile([128, D], f32, tag='oaps')
                nc.tensor.matmul(oaps[:], qT[:, 0:128], woaS[:, 0:D], start=True, stop=False)
                nc.tensor.matmul(oaps[:], qT[:, 128:256], woaS[:, D:2 * D], start=False, stop=False)
                nc.tensor.matmul(oaps[:], onesS[:1, :], boaS[:1, :], start=False, stop=True)

                # softmax over (lv,p) per h; cols (lv,p,h), h innermost
                ex = wk.tile([128, 128], f32, tag='ex')
                nc.scalar.activation(ex[:], oaps[:, 128:256], AF.Exp)
                zs = wk.tile([128, 8], f32, tag='zs')
                nc.vector.tensor_reduce(zs[:], _bc(ex[:, :], [[1, H], [H, 16]]),
                                        mybir.AxisListType.X, OP.add)
                zr = wk.tile([128, 8], f32, tag='zr')
                nc.vector.reciprocal(zr[:], zs[:])
                wnh = wk.tile([128, 128], fp16, tag='wnh')
                nc.vector.tensor_tensor(
                    _bc(wnh[:, :], [[H, 16], [1, H]]),
                    _bc(ex[:, :], [[H, 16], [1, H]]),
                    _bc(zr[:, :], [[0, 16], [1, H]]), OP.mult)

                # positions
                art = wk.tile([128, L], f32, tag='art')
                nc.vector.tensor_tensor(art[:], reft[:], tscS[:, :], OP.mult)
                ar16 = wk.tile([128, L], f32, tag='ar16')
                nc.vector.tensor_scalar(ar16[:], art[:], 15.5, None, OP.add)
                fli = wk.tile([128, L], i32, tag='fli')
                nc.vector.tensor_copy(fli[:], ar16[:])
                fl16 = wk.tile([128, L], f32, tag='fl16')
                nc.vector.tensor_copy(fl16[:], fli[:])
                wri = car.tile([128, L], i32, tag='wri')
                nc.vector.tensor_tensor(wri[:], fli[:], slcS[:, :], OP.add)
                arf = wk.tile([128, L], f32, tag='arf')
                nc.vector.scalar_tensor_tensor(arf[:], ar16[:], 2.0, fl16[:], OP.add, OP.subtract)

                xq = wk.tile([128, HLP], fp16, tag='xq')
                nc.vector.tensor_tensor(
                    xq[:], oaps[:, 0:128],
                    _bc(arf[:, :], [[1, L], [0, NP], [0, H]]), OP.add)

                dd = wk.tile([128, HLP * W], fp16, tag='dd', bufs=2)
                nc.vector.tensor_tensor(
                    dd[:], riotaS[:, :],
                    _bc(xq[:, :], [[H, 16], [0, W], [1, H]]), OP.subtract)
                da = wk.tile([128, HLP * W], fp16, tag='da', bufs=2)
                nc.scalar.activation(da[:], dd[:], AF.Abs)
                h1 = wk.tile([128, HLP * W], fp16, tag='h1', bufs=2)
                nc.scalar.activation(h1[:], da[:], AF.Relu, bias=1.0, scale=-1.0)
                c3p = wk.tile([128, HLP * W], fp16, tag='c3p')
                nc.vector.tensor_tensor(
                    c3p[:], h1[:],
                    _bc(wnh[:, :], [[H, 16], [0, W], [1, H]]), OP.mult)

                c3a = wk.tile([128, LRH], fp16, tag='c3a')
                nc.vector.tensor_tensor(
                    c3a[:],
                    _bc(c3p[:, :], [[4 * WH, L], [H, W], [1, H]]),
                    _bc(_off(c3p[:, :], WH), [[4 * WH, L], [H, W], [1, H]]), OP.add)
                c3b = wk.tile([128, LRH], fp16, tag='c3b')
                nc.vector.tensor_tensor(
                    c3b[:],
                    _bc(_off(c3p[:, :], 2 * WH), [[4 * WH, L], [H, W], [1, H]]),
                    _bc(_off(c3p[:, :], 3 * WH), [[4 * WH, L], [H, W], [1, H]]), OP.add)
                c3 = car.tile([128, LRH], fp16, tag='c3')
                nc.vector.tensor_tensor(c3[:], c3a[:], c3b[:], OP.add)

                s['wri'] = wri
                s['c3'] = c3

            # ---- G: issue gathers for tile i ----
            def G(i):
                s = st[i]
                gws = []
                for lv in range(L):
                    gw = gat.tile([128, WIN], bf16, tag=f'gw{lv}', bufs=3)
                    nc.gpsimd.indirect_dma_start(
                        out=gw[:], out_offset=None, in_=vwin,
                        in_offset=bass.IndirectOffsetOnAxis(ap=s['wri'][:, lv:lv + 1], axis=0))
                    gws.append(gw)
                s['gws'] = gws

            # ---- S1: sampling reduce + out-projection issue ----
            def S1(i):
                s = st[i]
                c3 = s['c3']
                prod = wk.tile([128, 4 * WIN], bf16, tag='prod', bufs=2)
                for lv in range(L):
                    nc.vector.tensor_tensor(
                        prod[:, lv * WIN:(lv + 1) * WIN],
                        s['gws'][lv][:],
                        _bc(_off(c3[:, :], lv * WH), [[H, W], [0, 32], [1, H]]),
                        OP.mult)
                s1 = wk.tile([128, 2 * WIN], bf16, tag='s1', bufs=2)
                nc.vector.tensor_tensor(s1[:], prod[:, 0:2 * WIN], prod[:, 2 * WIN:4 * WIN], OP.add)
                s2 = wk.tile([128, WIN], bf16, tag='s2', bufs=2)
                nc.vector.tensor_tensor(s2[:], s1[:, 0:WIN], s1[:, WIN:2 * WIN], OP.add)
                s3 = wk.tile([128, 3 * D], bf16, tag='s3')
                nc.vector.tensor_tensor(s3[:], s2[:, 0:3 * D], s2[:, 3 * D:6 * D], OP.add)
                a1 = wk.tile([128, D], bf16, tag='a1')
                nc.vector.tensor_tensor(a1[:], s3[:, 0:D], s3[:, D:2 * D], OP.add)
                att = wk.tile([128, D], bf16, tag='att')
                nc.vector.tensor_tensor(att[:], a1[:], s3[:, 2 * D:3 * D], OP.add)

                attT = wk.tile([128, D], bf16, tag='attT')
                for kc in range(2):
                    tp = ptr.tile([128, 128], bf16, tag='tr')
                    nc.tensor.transpose(tp[:], att[:, kc * 128:(kc + 1) * 128], identS[:])
                    nc.scalar.activation(attT[:, kc * 128:(kc + 1) * 128], tp[:], AF.Copy)
                s2ps = psB.tile([128, D], f32, tag='p512')
                nc.tensor.matmul(s2ps[:], attT[:, 0:128], woutS[:, 0:D], start=True, stop=False)
                nc.tensor.matmul(s2ps[:], attT[:, 128:256], woutS[:, D:2 * D], start=False, stop=False)
                nc.tensor.matmul(s2ps[:], onesS[:1, :], boutS[:1, :], start=False, stop=True)
                s['s2ps'] = s2ps

            # ---- S2: LN1 + FFN issue ----
            def S2(i):
                rq = i * 128
                s = st[i]
                srco = io.tile([128, D], f32, tag='srco2')
                nc.sync.dma_start(srco[:], srcq[rq:rq + 128, :])
                tet = io.tile([128, D], f32, tag='tet')
                nc.sync.dma_start(tet[:], te_q[rq:rq + 128, :])

                sfull = wk.tile([128, D], f32, tag='sfull')
                nc.vector.tensor_tensor(sfull[:], srco[:], s['s2ps'][:], OP.add)
                bns = wk.tile([128, 6], f32, tag='bns')
                nc.vector.bn_stats(bns[:], sfull[:])
                mv = wk.tile([128, 2], f32, tag='mv')
                nc.vector.bn_aggr(mv[:], bns[:])
                sd = wk.tile([128, 1], f32, tag='sd')
                nc.scalar.activation(sd[:], mv[:, 1:2], AF.Sqrt, bias=epsS[:, 0:1])
                rsd = wk.tile([128, 1], f32, tag='rsd')
                nc.vector.reciprocal(rsd[:], sd[:])
                nmr = wk.tile([128, 1], f32, tag='nmr')
                nc.vector.scalar_tensor_tensor(nmr[:], mv[:, 0:1], -1.0, rsd[:], OP.mult, OP.mult)
                xn = wk.tile([128, D], bf16, tag='xn')
                nc.scalar.activation(xn[:], sfull[:], AF.Identity, scale=rsd[:, 0:1], bias=nmr[:, 0:1])
                teb = wk.tile([128, D], f32, tag='teb')
                nc.gpsimd.tensor_tensor(teb[:], tet[:], ln1bS[:, :], OP.add)
                t1 = wk.tile([128, D], bf16, tag='t1')
                nc.gpsimd.tensor_tensor(t1[:], xn[:], ln1gS[:, :], OP.mult)
                xx = wk.tile([128, D], bf16, tag='xx', bufs=5)
                nc.vector.tensor_tensor(xx[:], t1[:], teb[:], OP.add)

                xT = wk.tile([128, D], bf16, tag='xT')
                for kc in range(2):
                    tp = ptr.tile([128, 128], bf16, tag='tr')
                    nc.tensor.transpose(tp[:], xx[:, kc * 128:(kc + 1) * 128], identS[:])
                    nc.scalar.activation(xT[:, kc * 128:(kc + 1) * 128], tp[:], AF.Copy)
                hbT = wk.tile([128, DF], bf16, tag='hbT', bufs=2)
                for k in range(8):
                    hps = ptr.tile([128, 128], f32, tag='tr')
                    for kc in range(2):
                        nc.tensor.matmul(hps[:], w1S[:, kc * DF + k * 128: kc * DF + (k + 1) * 128],
                                         xT[:, kc * 128:(kc + 1) * 128],
                                         start=(kc == 0), stop=(kc == 1))
                    nc.scalar.activation(hbT[:, k * 128:(k + 1) * 128], hps[:], AF.Relu,
                                         bias=b1tS[:, k:k + 1])
                o2ps = psA.tile([128, D], f32, tag='oaps')
                for k in range(8):
                    nc.tensor.matmul(o2ps[:], hbT[:, k * 128:(k + 1) * 128],
                                     w2S[:, k * D:(k + 1) * D],
                                     start=(k == 0), stop=False)
                nc.tensor.matmul(o2ps[:], onesS[:1, :], b2S[:1, :], start=False, stop=True)
                s['xx'] = xx
                s['o2ps'] = o2ps

            # ---- S3: LN2 + store ----
            def S3(i):
                rq = i * 128
                s = st[i]
                sf2 = wk.tile([128, D], f32, tag='sf2')
                nc.vector.tensor_tensor(sf2[:], s['xx'][:], s['o2ps'][:], OP.add)
                bns2 = wk.tile([128, 6], f32, tag='bns2')
                nc.vector.bn_stats(bns2[:], sf2[:])
                mv2 = wk.tile([128, 2], f32, tag='mv2')
                nc.vector.bn_aggr(mv2[:], bns2[:])
                sd2 = wk.tile([128, 1], f32, tag='sd2')
                nc.scalar.activation(sd2[:], mv2[:, 1:2], AF.Sqrt, bias=epsS[:, 0:1])
                rsd2 = wk.tile([128, 1], f32, tag='rsd2')
                nc.vector.reciprocal(rsd2[:], sd2[:])
                nmr2 = wk.tile([128, 1], f32, tag='nmr2')
                nc.vector.scalar_tensor_tensor(nmr2[:], mv2[:, 0:1], -1.0, rsd2[:], OP.mult, OP.mult)
                xn2 = wk.tile([128, D], bf16, tag='xn2')
                nc.scalar.activation(xn2[:], sf2[:], AF.Identity, scale=rsd2[:, 0:1], bias=nmr2[:, 0:1])
                t2 = wk.tile([128, D], f32, tag='t2')
                nc.gpsimd.tensor_tensor(t2[:], xn2[:], ln2gS[:, :], OP.mult)
                of = wk.tile([128, D], f32, tag='of')
                nc.gpsimd.tensor_tensor(of[:], t2[:], ln2bS[:, :], OP.add)
                nc.sync.dma_start(out_q[rq:rq + 128, :], of[:])

            # ---- emission schedule ----
            NQUAD = LEN_IN // 512  # 30
            n_tiles = int(os.environ.get('K_NTQ', NT_Q))
            for j in range(NQUAD):
                phaseA_quad(j)
                if j < n_tiles:
                    S0(j)
            for i in range(NQUAD, n_tiles):
                S0(i)

            G(0)
            for i in range(n_tiles):
                if i + 1 < n_tiles:
                    G(i + 1)
                S1(i)
                if i >= 2:
                    S3(i - 2)
                S2(i)
            if n_tiles >= 2:
                S3(n_tiles - 2)
            S3(n_tiles - 1)

    if os.environ.get('K_NOSPLIT', '0') != '1':
        _split_excess_waits(nc)
    return nc


_PROG = None
LAST_RESULTS = None


def _get_program():
    global _PROG
    if _PROG is None:
        _PROG = _build_program()
    return _PROG


# host-side layout permutations
_PERM_DH = np.array([h * 32 + d for d in range(32) for h in range(H)])      # value dims
_PERM_LPH = np.array([h * 16 + l * 4 + p                                    # off/attn cols
                      for l in range(L) for p in range(NP) for h in range(H)])


def _host_consts():
    bfc = lambda a: np.ascontiguousarray(np.asarray(a, np.float32)).astype(ml_dtypes.bfloat16)
    c = {}
    c['ident'] = bfc(np.eye(128, dtype=np.float32))
    c['ones1'] = bfc(np.ones((1, 128), np.float32))
    ri = np.zeros((HLP * W,), np.float32)
    for lvp in range(16):
        for r in range(W):
            for h in range(H):
                ri[lvp * 48 + r * 8 + h] = r
    c['riota'] = np.repeat(ri[None, :], 128, axis=0).astype(np.float16)
    c['tsc'] = np.repeat(np.array([LEVEL_LENS], np.float32), 128, axis=0)
    c['slc'] = np.repeat(np.array([[PSTARTS[lv] - 18 for lv in range(L)]], np.int32),
                         128, axis=0)
    return c


def kernel(src, pos, time_embed, reference_points, w_off, b_off, w_attn, b_attn,
           w_val, b_val, w_out, b_out, ln1_g, ln1_b, w1, b1, w2, b2, ln2_g, ln2_b,
           spatial_shapes, level_start_index):
    src = np.asarray(src, np.float32)
    pos = np.asarray(pos, np.float32)
    te = np.asarray(time_embed, np.float32)
    ref = np.asarray(reference_points, np.float32).reshape(2, LEN_IN, L)

    bfc = lambda a: np.ascontiguousarray(np.asarray(a, np.float32)).astype(ml_dtypes.bfloat16)
    consts = _host_consts()

    def chunk2(w):  # [256, X] -> [128, 2X]
        w = np.asarray(w, np.float32)
        return np.concatenate([w[0:128, :], w[128:256, :]], axis=1)

    woa_full = np.concatenate([np.asarray(w_off, np.float32)[:, _PERM_LPH],
                               np.asarray(w_attn, np.float32)[:, _PERM_LPH]], axis=1)
    boa_full = np.concatenate([np.asarray(b_off, np.float32)[_PERM_LPH],
                               np.asarray(b_attn, np.float32)[_PERM_LPH]])[None, :]

    wval_p = np.asarray(w_val, np.float32)[:, _PERM_DH]
    bval_p = np.asarray(b_val, np.float32)[_PERM_DH][None, :]
    bval_p2 = np.concatenate([bval_p, bval_p], axis=1)  # [1, 512] for N=512 bias mm
    wout_p = np.asarray(w_out, np.float32)[_PERM_DH, :]

    w2f = np.asarray(w2, np.float32)
    w2c = np.concatenate([w2f[k * 128:(k + 1) * 128, :] for k in range(8)], axis=1)
    b1f = np.asarray(b1, np.float32)
    b1t = np.stack([b1f[k * 128:(k + 1) * 128] for k in range(8)], axis=1)  # [128, 8]
    rep = lambda v: np.repeat(np.asarray(v, np.float32)[None, :], 128, axis=0)

    shared = {
        'wval': bfc(chunk2(wval_p)),
        'bval': bfc(bval_p2),
        'woa': bfc(chunk2(woa_full)),
        'boa': bfc(boa_full),
        'wout': bfc(chunk2(wout_p)),
        'bout': bfc(np.asarray(b_out, np.float32)[None, :]),
        'w1': bfc(chunk2(np.asarray(w1, np.float32))),
        'b1t': b1t.astype(np.float32),
        'w2': bfc(w2c),
        'b2': bfc(np.asarray(b2, np.float32)[None, :]),
        'ln1g': rep(ln1_g).astype(ml_dtypes.bfloat16), 'ln1b': rep(ln1_b),
        'ln2g': rep(ln2_g).astype(ml_dtypes.bfloat16), 'ln2b': rep(ln2_b),
        **consts,
    }

    in_maps = []
    for c in range(N_CORES):
        n, qr = c // 4, c % 4
        m = dict(shared)
        m['srcT'] = np.ascontiguousarray(src[n].T).astype(ml_dtypes.bfloat16)
        m['srcq'] = src[n, qr * QC:(qr + 1) * QC]
        m['pos_q'] = pos[n, qr * QC:(qr + 1) * QC]
        m['te_q'] = te[n, qr * QC:(qr + 1) * QC]
        m['ref_q'] = ref[n, qr * QC:(qr + 1) * QC]
        in_maps.append(m)

    nc = _get_program()
    from concourse.bass_utils import run_bass_kernel_spmd
    res = run_bass_kernel_spmd(nc, in_maps, core_ids=list(range(N_CORES)))
    global LAST_RESULTS
    LAST_RESULTS = res
    if getattr(res, 'exec_time_ns', None):
        print('HW exec time:', res.exec_time_ns, 'ns')

    out = np.zeros((2, LEN_IN, D), np.float32)
    for c in range(N_CORES):
        n, qr = c // 4, c % 4
        out[n, qr * QC:(qr + 1) * QC] = res.results[c]['out_q']
    return out
